# revision 1
# baseline (speedup 1.0000x reference)
"""Trainium2 Bass kernel for nn_CrossAttensionFusion2D.

Data-parallel over batch: core b computes batch element b end-to-end (no
collectives). Per-core pipeline:

  P1  six 1x1 convs as matmuls (channel-padded layouts, bias via K=1 matmul)
  P2  PE-transpose of v/v_bpf to token-major (AV stationary operand), with a
      ones column baked in at dp=12 to produce softmax denominators for free
  P3  attention: per 4-head group, row-tiled K=16 QK^T matmuls (4 heads
      concurrently in the 4 PE row-groups, one PSUM bank each), one ACT Exp
      per head-pair straight from PSUM into bf16 SBUF, then col-tiled AV
      matmuls accumulating [32,512] strips per head in a shared PSUM bank
  P4  denominators: strided-partition copy from AV PSUM, vector reciprocal,
      selector matmul broadcasts 1/den to each head's 12 channel rows, one
      in-place tensor_mul normalizes
  P5  output conv over the padded fused channels + residual terms as extra
      accumulation over the original x/x_bpf, bias via K=1 matmul

Channel layouts (host-side weight shuffles make the kernel layouts free):
  q/k space: head h at partitions 32*(h%4) of tile h//4, dims dp<16 (12 real)
  v space:   head h at columns 32*h..32*h+32 of token-major vT (12 real + ones)
"""
import numpy as np
import ml_dtypes
from contextlib import ExitStack

import concourse.bass as bass
import concourse.tile as tile
from concourse import bacc, mybir
from concourse.bass_utils import run_bass_kernel_spmd

f32 = mybir.dt.float32
bf16 = mybir.dt.bfloat16

B, EMBED, HIDDEN, NH, D, H, W = 8, 384, 512, 32, 12, 32, 32
HW = H * W  # 1024 tokens
SCALE = float(D) ** -0.5
NCORES = 8
NGRP = 8  # head groups per branch, 4 heads each


def _qk_row(h, dp):
    # padded q/k channel index: tile h//4, 32-aligned slot h%4, dim dp
    return 128 * (h // 4) + 32 * (h % 4) + dp


def _v_col(h, dp):
    return 32 * h + dp


def build_program(reps=1):
    nc = bacc.Bacc("TRN2", target_bir_lowering=False, debug=False)

    x_d = nc.dram_tensor("x", [EMBED, HW], bf16, kind="ExternalInput").ap()
    xb_d = nc.dram_tensor("xb", [EMBED, HW], bf16, kind="ExternalInput").ap()
    wq_d = nc.dram_tensor("wq", [EMBED, 1024], bf16, kind="ExternalInput").ap()
    wk_d = nc.dram_tensor("wk", [EMBED, 1024], bf16, kind="ExternalInput").ap()
    wqb_d = nc.dram_tensor("wqb", [EMBED, 1024], bf16, kind="ExternalInput").ap()
    wkb_d = nc.dram_tensor("wkb", [EMBED, 1024], bf16, kind="ExternalInput").ap()
    wv_d = nc.dram_tensor("wv", [EMBED, 1024], bf16, kind="ExternalInput").ap()
    wvb_d = nc.dram_tensor("wvb", [EMBED, 1024], bf16, kind="ExternalInput").ap()
    bias_d = nc.dram_tensor("biases", [6, 1024], bf16, kind="ExternalInput").ap()
    wo_d = nc.dram_tensor("wo", [2048, HIDDEN], bf16, kind="ExternalInput").ap()
    wores_d = nc.dram_tensor("wores", [2 * EMBED, HIDDEN], bf16, kind="ExternalInput").ap()
    bo_d = nc.dram_tensor("bo", [1, HIDDEN], bf16, kind="ExternalInput").ap()
    sel_d = nc.dram_tensor("sel", [NGRP, 32, 128], f32, kind="ExternalInput").ap()
    ident_d = nc.dram_tensor("ident", [128, 128], bf16, kind="ExternalInput").ap()
    out_d = nc.dram_tensor("out", [HIDDEN, HW], f32, kind="ExternalOutput").ap()

    with tile.TileContext(nc) as tc, ExitStack() as ctx:
        # ---- whole-kernel pool: x, vT, constants (~47KB/partition) ----
        sb = ctx.enter_context(tc.tile_pool(name="sb", bufs=1))
        x_sb = [sb.tile([128, HW], bf16, tag=f"x{k}", name=f"x{k}") for k in range(3)]
        xb_sb = [sb.tile([128, HW], bf16, tag=f"xb{k}", name=f"xb{k}") for k in range(3)]
        vT_sb = [sb.tile([128, 1024], bf16, tag=f"vT{j}", name=f"vT{j}") for j in range(8)]
        vbT_sb = [sb.tile([128, 1024], bf16, tag=f"vbT{j}", name=f"vbT{j}") for j in range(8)]
        bias_sb = [sb.tile([1, 1024], bf16, tag=f"bias{i}", name=f"bias{i}") for i in range(6)]
        bo_sb = sb.tile([1, HIDDEN], bf16, tag="bo", name="bo")
        ones_sb = sb.tile([1, HW], bf16, tag="ones", name="ones")
        ident_sb = sb.tile([128, 128], bf16, tag="ident", name="ident")

        for k in range(3):
            nc.sync.dma_start(x_sb[k][:], x_d[128 * k : 128 * k + 128, :])
            nc.sync.dma_start(xb_sb[k][:], xb_d[128 * k : 128 * k + 128, :])
        for i in range(6):
            nc.sync.dma_start(bias_sb[i][:], bias_d[i : i + 1, :])
        nc.sync.dma_start(bo_sb[:], bo_d[:])
        nc.sync.dma_start(ident_sb[:], ident_d[:])
        nc.vector.memset(ones_sb[:], 1.0)

        # ---- P3..P5 outputs: num + den (~40KB/partition) ----
        post = ctx.enter_context(tc.tile_pool(name="post", bufs=1))
        num_sb = [post.tile([128, HW], bf16, tag=f"num{t}", name=f"num{t}") for t in range(16)]
        den_bf = [post.tile([32, HW], bf16, tag=f"denb{b}", name=f"denb{b}") for b in range(2)]
        den_f32 = [post.tile([32, HW], f32, tag=f"den{b}", name=f"den{b}") for b in range(2)]

        for _rep in range(reps):
            # ---- q/k conv outputs, live P1..P3 (~64KB/partition) ----
            with ExitStack() as qctx:
                qk = qctx.enter_context(tc.tile_pool(name="qk", bufs=1))
                q_sb = [qk.tile([128, HW], bf16, tag=f"q{g}", name=f"q{g}") for g in range(NGRP)]
                k_sb = [qk.tile([128, HW], bf16, tag=f"k{g}", name=f"k{g}") for g in range(NGRP)]
                qb_sb = [qk.tile([128, HW], bf16, tag=f"qb{g}", name=f"qb{g}") for g in range(NGRP)]
                kb_sb = [qk.tile([128, HW], bf16, tag=f"kb{g}", name=f"kb{g}") for g in range(NGRP)]

                def load_w(pool, wd, name):
                    tiles = [pool.tile([128, 1024], bf16, tag=f"{name}{k}", name=f"{name}{k}")
                             for k in range(3)]
                    for k in range(3):
                        nc.sync.dma_start(tiles[k][:], wd[128 * k : 128 * k + 128, :])
                    return tiles

                def conv_tile(wtiles, bias_row, src, g, mm):
                    for nhf in range(2):
                        sl = slice(512 * nhf, 512 * nhf + 512)
                        for k in range(3):
                            nc.tensor.matmul(
                                mm[:, sl], wtiles[k][:, 128 * g : 128 * g + 128],
                                src[k][:, sl], start=(k == 0), stop=False)
                        nc.tensor.matmul(
                            mm[:, sl], bias_sb[bias_row][0:1, 128 * g : 128 * g + 128],
                            ones_sb[0:1, sl], start=False, stop=True)

                # P1 wave 1: q, k from x
                with ExitStack() as pctx:
                    wsb = pctx.enter_context(tc.tile_pool(name="wsb1", bufs=1))
                    ps_conv = pctx.enter_context(
                        tc.tile_pool(name="ps_conv1", bufs=2, space="PSUM"))
                    wq_t = load_w(wsb, wq_d, "wq")
                    wk_t = load_w(wsb, wk_d, "wk")
                    for g in range(8):
                        mm = ps_conv.tile([128, HW], f32, tag="convmm", name="convmm")
                        conv_tile(wq_t, 0, x_sb, g, mm)
                        nc.vector.tensor_copy(q_sb[g][:], mm[:])
                    for g in range(8):
                        mm = ps_conv.tile([128, HW], f32, tag="convmm", name="convmm")
                        conv_tile(wk_t, 1, x_sb, g, mm)
                        nc.vector.tensor_copy(k_sb[g][:], mm[:])

                # P1 wave 2: q_bpf, k_bpf from x_bpf
                with ExitStack() as pctx:
                    wsb = pctx.enter_context(tc.tile_pool(name="wsb2", bufs=1))
                    ps_conv = pctx.enter_context(
                        tc.tile_pool(name="ps_conv2", bufs=2, space="PSUM"))
                    wqb_t = load_w(wsb, wqb_d, "wqb")
                    wkb_t = load_w(wsb, wkb_d, "wkb")
                    for g in range(8):
                        mm = ps_conv.tile([128, HW], f32, tag="convmm", name="convmm")
                        conv_tile(wqb_t, 2, xb_sb, g, mm)
                        nc.vector.tensor_copy(qb_sb[g][:], mm[:])
                    for g in range(8):
                        mm = ps_conv.tile([128, HW], f32, tag="convmm", name="convmm")
                        conv_tile(wkb_t, 3, xb_sb, g, mm)
                        nc.vector.tensor_copy(kb_sb[g][:], mm[:])

                # P2: v convs + transposes
                with ExitStack() as pctx:
                    wsb = pctx.enter_context(tc.tile_pool(name="wsb3", bufs=1))
                    ps_conv = pctx.enter_context(
                        tc.tile_pool(name="ps_conv3", bufs=2, space="PSUM"))
                    ps_tr = pctx.enter_context(
                        tc.tile_pool(name="ps_tr", bufs=2, space="PSUM"))
                    vstage = pctx.enter_context(tc.tile_pool(name="vstage", bufs=1))

                    def vconv_transpose(wtiles, bias_row, src, vT_tiles):
                        # wave w: conv out tiles tau in [4w,4w+4) -> vT cols 512w..+512
                        for w in range(2):
                            vtmp = [vstage.tile([128, 1024], bf16, tag=f"vs{t}", name=f"vs{t}")
                                    for t in range(4)]
                            for t in range(4):
                                g = 4 * w + t
                                mm = ps_conv.tile([128, HW], f32, tag="convmm", name="convmm")
                                conv_tile(wtiles, bias_row, src, g, mm)
                                nc.vector.tensor_copy(vtmp[t][:], mm[:])
                            for j in range(8):
                                tp = ps_tr.tile([128, 512], bf16, tag="tp", name="tp")
                                for t in range(4):
                                    nc.tensor.transpose(
                                        tp[:, 128 * t : 128 * t + 128],
                                        vtmp[t][:, 128 * j : 128 * j + 128], ident_sb[:])
                                nc.vector.tensor_copy(
                                    vT_tiles[j][:, 512 * w : 512 * w + 512], tp[:])
                        for j in range(8):
                            # ones column at dp=12 of every head -> denominator channel
                            nc.vector.memset(vT_tiles[j][:, 12:1024:32], 1.0)

                    wv_t = load_w(wsb, wv_d, "wv")
                    wvb_t = load_w(wsb, wvb_d, "wvb")
                    vconv_transpose(wv_t, 4, x_sb, vT_sb)
                    vconv_transpose(wvb_t, 5, xb_sb, vbT_sb)

                # ---- P3: attention ----
                with ExitStack() as pctx:
                    ps_sc = pctx.enter_context(tc.tile_pool(name="ps_sc", bufs=2, space="PSUM"))
                    ps_av = pctx.enter_context(tc.tile_pool(name="ps_av", bufs=2, space="PSUM"))
                    peT = pctx.enter_context(tc.tile_pool(name="peT", bufs=3))

                    for br in range(2):
                        # br0: f = mha(q_bpf, k, v);  br1: f_bpf = mha(q, k_bpf, v_bpf)
                        qs = qb_sb if br == 0 else q_sb
                        ks = k_sb if br == 0 else kb_sb
                        vTs = vT_sb if br == 0 else vbT_sb
                        for grp in range(NGRP):
                            t16 = 8 * br + grp
                            for nh in range(2):
                                av = ps_av.tile([128, 512], f32, tag="av", name="av")
                                for m in range(8):
                                    for p in range(2):
                                        s2 = ps_sc.tile([128, 1024], f32, tag="s2", name="s2")
                                        for cl in range(2):
                                            c = 2 * p + cl
                                            nc.tensor.matmul(
                                                s2[:, 512 * cl : 512 * cl + 512],
                                                ks[grp][32 * c : 32 * c + 16, 128 * m : 128 * m + 128],
                                                qs[grp][32 * c : 32 * c + 16, 512 * nh : 512 * nh + 512],
                                                start=True, stop=True, tile_position=(32 * c, 0))
                                        eT = peT.tile([128, 1024], bf16, tag="eT", name="eT")
                                        nc.scalar.activation(
                                            eT[:], s2[:], mybir.ActivationFunctionType.Exp)
                                        for cl in range(2):
                                            c = 2 * p + cl
                                            h = 4 * grp + c
                                            nc.tensor.matmul(
                                                av[32 * c : 32 * c + 32, :],
                                                vTs[m][:, 32 * h : 32 * h + 32],
                                                eT[:, 512 * cl : 512 * cl + 512],
                                                start=(m == 0), stop=(m == 7),
                                                tile_position=(0, 32 * c),
                                                skip_group_check=True)
                                # numerators (bf16); denominator rows gathered by DMA
                                # (compute engines cannot read strided partitions)
                                nc.vector.tensor_copy(
                                    num_sb[t16][:, 512 * nh : 512 * nh + 512], av[:])
                                for c in range(4):
                                    nc.sync.dma_start(
                                        den_bf[br][4 * grp + c : 4 * grp + c + 1,
                                                   512 * nh : 512 * nh + 512],
                                        num_sb[t16][32 * c + 12 : 32 * c + 13,
                                                    512 * nh : 512 * nh + 512])

            # ---- P4: normalize (q/k pool closed above frees SBUF) ----
            with ExitStack() as pctx:
                p4 = pctx.enter_context(tc.tile_pool(name="p4", bufs=1))
                ps_rdb = pctx.enter_context(tc.tile_pool(name="ps_rdb", bufs=2, space="PSUM"))
                rd_f32 = [p4.tile([32, HW], f32, tag=f"rd{b}", name=f"rd{b}") for b in range(2)]
                sel_sb = [p4.tile([32, 128], f32, tag=f"sel{g}", name=f"sel{g}") for g in range(NGRP)]
                for g in range(NGRP):
                    nc.sync.dma_start(sel_sb[g][:], sel_d[g])
                for br in range(2):
                    nc.vector.tensor_copy(den_f32[br][:], den_bf[br][:])
                    nc.vector.reciprocal(rd_f32[br][:], den_f32[br][:])
                for br in range(2):
                    for grp in range(NGRP):
                        t16 = 8 * br + grp
                        rdb = ps_rdb.tile([128, HW], f32, tag="rdb", name="rdb")
                        for nh in range(2):
                            nc.tensor.matmul(
                                rdb[:, 512 * nh : 512 * nh + 512], sel_sb[grp][:],
                                rd_f32[br][:, 512 * nh : 512 * nh + 512],
                                start=True, stop=True)
                        nc.vector.tensor_mul(num_sb[t16][:], num_sb[t16][:], rdb[:])

            # ---- P5: output conv (+ residual terms over x/x_bpf) ----
            with ExitStack() as pctx:
                p5 = pctx.enter_context(tc.tile_pool(name="p5", bufs=1))
                ps_out = pctx.enter_context(tc.tile_pool(name="ps_out", bufs=2, space="PSUM"))
                po = pctx.enter_context(tc.tile_pool(name="po", bufs=2))
                wo_sb = [p5.tile([128, HIDDEN], bf16, tag=f"wo{t}", name=f"wo{t}") for t in range(16)]
                wores_sb = [p5.tile([128, HIDDEN], bf16, tag=f"wor{k}", name=f"wor{k}") for k in range(6)]
                for t in range(16):
                    nc.sync.dma_start(wo_sb[t][:], wo_d[128 * t : 128 * t + 128, :])
                for k in range(6):
                    nc.sync.dma_start(wores_sb[k][:], wores_d[128 * k : 128 * k + 128, :])
                for tau in range(4):
                    mm = ps_out.tile([128, HW], f32, tag="omm", name="omm")
                    for nhf in range(2):
                        sl = slice(512 * nhf, 512 * nhf + 512)
                        for t16 in range(16):
                            nc.tensor.matmul(
                                mm[:, sl], wo_sb[t16][:, 128 * tau : 128 * tau + 128],
                                num_sb[t16][:, sl], start=(t16 == 0), stop=False)
                        for k in range(6):
                            rhs = x_sb[k] if k < 3 else xb_sb[k - 3]
                            nc.tensor.matmul(
                                mm[:, sl], wores_sb[k][:, 128 * tau : 128 * tau + 128],
                                rhs[:, sl], start=False, stop=False)
                        nc.tensor.matmul(
                            mm[:, sl], bo_sb[0:1, 128 * tau : 128 * tau + 128],
                            ones_sb[0:1, sl], start=False, stop=True)
                    o_sb = po.tile([128, HW], f32, tag="osb", name="osb")
                    nc.vector.tensor_copy(o_sb[:], mm[:])
                    nc.sync.dma_start(out_d[128 * tau : 128 * tau + 128, :], o_sb[:])

    nc.compile()
    return nc


def prep_weights(Wq, bq, Wk, bk, Wv, bv, Wq_bpf, bq_bpf, Wk_bpf, bk_bpf,
                 Wv_bpf, bv_bpf, Wo, bo):
    """Host-side channel shuffles + scale folding. Returns dict of arrays."""
    def qk_pad(Wm, bias, scale):
        wt = np.zeros((EMBED, 1024), np.float32)
        bp = np.zeros((1024,), np.float32)
        for h in range(NH):
            r0 = _qk_row(h, 0)
            wt[:, r0 : r0 + D] = (Wm[12 * h : 12 * h + 12, :] * scale).T
            bp[r0 : r0 + D] = bias[12 * h : 12 * h + 12] * scale
        return wt, bp

    def v_pad(Wm, bias):
        wt = np.zeros((EMBED, 1024), np.float32)
        bp = np.zeros((1024,), np.float32)
        for h in range(NH):
            r0 = _v_col(h, 0)
            wt[:, r0 : r0 + D] = Wm[12 * h : 12 * h + 12, :].T
            bp[r0 : r0 + D] = bias[12 * h : 12 * h + 12]
        return wt, bp

    wq_t, bq_p = qk_pad(Wq, bq, SCALE)        # branch-2 query (scaled)
    wk_t, bk_p = qk_pad(Wk, bk, 1.0)
    wqb_t, bqb_p = qk_pad(Wq_bpf, bq_bpf, SCALE)  # branch-1 query (scaled)
    wkb_t, bkb_p = qk_pad(Wk_bpf, bk_bpf, 1.0)
    wv_t, bv_p = v_pad(Wv, bv)
    wvb_t, bvb_p = v_pad(Wv_bpf, bv_bpf)

    biases = np.stack([bq_p, bk_p, bqb_p, bkb_p, bv_p, bvb_p])  # [6, 1024]

    # output conv over padded fused channels
    wo_big = np.zeros((2048, HIDDEN), np.float32)
    for br in range(2):
        for h in range(NH):
            grp, c = h // 4, h % 4
            row0 = 128 * (8 * br + grp) + 32 * c
            col0 = EMBED * br + 12 * h
            wo_big[row0 : row0 + D, :] = Wo[:, col0 : col0 + D].T
    wo_res = Wo.T.copy()  # [768, 512]; rows 0-383 pair with x, 384-767 with x_bpf

    sel = np.zeros((NGRP, 32, 128), np.float32)
    for grp in range(NGRP):
        for c in range(4):
            h = 4 * grp + c
            for dp in range(D):
                sel[grp, h, 32 * c + dp] = 1.0

    bf = ml_dtypes.bfloat16
    return {
        "wq": wq_t.astype(bf), "wk": wk_t.astype(bf),
        "wqb": wqb_t.astype(bf), "wkb": wkb_t.astype(bf),
        "wv": wv_t.astype(bf), "wvb": wvb_t.astype(bf),
        "biases": biases.astype(bf),
        "wo": wo_big.astype(bf), "wores": wo_res.astype(bf),
        "bo": bo.reshape(1, HIDDEN).astype(bf),
        "sel": sel,
        "ident": np.eye(128).astype(bf),
    }


_NC = None


def kernel(x, x_bpf, Wq, bq, Wk, bk, Wv, bv, Wq_bpf, bq_bpf, Wk_bpf, bk_bpf,
           Wv_bpf, bv_bpf, Wo, bo, _trace=False):
    global _NC
    if _NC is None:
        _NC = build_program()
    nc = _NC

    shared = prep_weights(np.asarray(Wq, np.float32), np.asarray(bq, np.float32),
                          np.asarray(Wk, np.float32), np.asarray(bk, np.float32),
                          np.asarray(Wv, np.float32), np.asarray(bv, np.float32),
                          np.asarray(Wq_bpf, np.float32), np.asarray(bq_bpf, np.float32),
                          np.asarray(Wk_bpf, np.float32), np.asarray(bk_bpf, np.float32),
                          np.asarray(Wv_bpf, np.float32), np.asarray(bv_bpf, np.float32),
                          np.asarray(Wo, np.float32), np.asarray(bo, np.float32))

    bf = ml_dtypes.bfloat16
    x = np.asarray(x, np.float32).reshape(B, EMBED, HW)
    x_bpf = np.asarray(x_bpf, np.float32).reshape(B, EMBED, HW)
    in_maps = []
    for b in range(B):
        m = dict(shared)
        m["x"] = x[b].astype(bf)
        m["xb"] = x_bpf[b].astype(bf)
        in_maps.append(m)

    res = run_bass_kernel_spmd(nc, in_maps, list(range(NCORES)), trace=_trace)
    out = np.stack([res.results[b]["out"] for b in range(B)])  # [8, 512, 1024]
    out = out.reshape(B, HIDDEN, H, W).astype(np.float32)
    if _trace:
        return out, res
    return out



# revision 8
# speedup vs baseline: 1.0256x; 1.0256x over previous
"""Trainium2 Bass kernel for nn_CrossAttensionFusion2D.

Data-parallel over batch: core b computes batch element b end-to-end (no
collectives). Per-core pipeline:

  P1  six 1x1 convs as matmuls (channel-padded layouts, bias via K=1 matmul)
  P2  PE-transpose of v/v_bpf to token-major (AV stationary operand), with a
      ones column baked in at dp=12 to produce softmax denominators for free
  P3  attention: per 4-head group, row-tiled K=16 QK^T matmuls (4 heads
      concurrently in the 4 PE row-groups, one PSUM bank each), one ACT Exp
      per head-pair straight from PSUM into bf16 SBUF, then col-tiled AV
      matmuls accumulating [32,512] strips per head in a shared PSUM bank
  P4  denominators: strided-partition copy from AV PSUM, vector reciprocal,
      selector matmul broadcasts 1/den to each head's 12 channel rows, one
      in-place tensor_mul normalizes
  P5  output conv over the padded fused channels + residual terms as extra
      accumulation over the original x/x_bpf, bias via K=1 matmul

Channel layouts (host-side weight shuffles make the kernel layouts free):
  q/k space: head h at partitions 32*(h%4) of tile h//4, dims dp<16 (12 real)
  v space:   head h at columns 32*h..32*h+32 of token-major vT (12 real + ones)
"""
import numpy as np
import ml_dtypes
from contextlib import ExitStack

import concourse.bass as bass
import concourse.tile as tile
from concourse import bacc, mybir
from concourse.bass_utils import run_bass_kernel_spmd

f32 = mybir.dt.float32
bf16 = mybir.dt.bfloat16

B, EMBED, HIDDEN, NH, D, H, W = 8, 384, 512, 32, 12, 32, 32
HW = H * W  # 1024 tokens
SCALE = float(D) ** -0.5
NCORES = 8
NGRP = 8  # head groups per branch, 4 heads each


def _qk_row(h, dp):
    # padded q/k channel index: tile h//4, 32-aligned slot h%4, dim dp
    return 128 * (h // 4) + 32 * (h % 4) + dp


def _v_col(h, dp):
    return 32 * h + dp


def build_program(reps=1):
    nc = bacc.Bacc("TRN2", target_bir_lowering=False, debug=False)

    x_d = nc.dram_tensor("x", [EMBED, HW], bf16, kind="ExternalInput").ap()
    xb_d = nc.dram_tensor("xb", [EMBED, HW], bf16, kind="ExternalInput").ap()
    wq_d = nc.dram_tensor("wq", [EMBED, 1024], bf16, kind="ExternalInput").ap()
    wk_d = nc.dram_tensor("wk", [EMBED, 1024], bf16, kind="ExternalInput").ap()
    wqb_d = nc.dram_tensor("wqb", [EMBED, 1024], bf16, kind="ExternalInput").ap()
    wkb_d = nc.dram_tensor("wkb", [EMBED, 1024], bf16, kind="ExternalInput").ap()
    wv_d = nc.dram_tensor("wv", [EMBED, 1024], bf16, kind="ExternalInput").ap()
    wvb_d = nc.dram_tensor("wvb", [EMBED, 1024], bf16, kind="ExternalInput").ap()
    bias_d = nc.dram_tensor("biases", [6, 1024], bf16, kind="ExternalInput").ap()
    wo_d = nc.dram_tensor("wo", [2048, HIDDEN], bf16, kind="ExternalInput").ap()
    wores_d = nc.dram_tensor("wores", [2 * EMBED, HIDDEN], bf16, kind="ExternalInput").ap()
    bo_d = nc.dram_tensor("bo", [1, HIDDEN], bf16, kind="ExternalInput").ap()
    sel_d = nc.dram_tensor("sel", [NGRP, 32, 128], f32, kind="ExternalInput").ap()
    ident_d = nc.dram_tensor("ident", [128, 128], bf16, kind="ExternalInput").ap()
    out_d = nc.dram_tensor("out", [HIDDEN, HW], f32, kind="ExternalOutput").ap()

    with tile.TileContext(nc) as tc, ExitStack() as ctx:
        # ---- whole-kernel pool: x, vT, constants (~47KB/partition) ----
        sb = ctx.enter_context(tc.tile_pool(name="sb", bufs=1))
        x_sb = [sb.tile([128, HW], bf16, tag=f"x{k}", name=f"x{k}") for k in range(3)]
        xb_sb = [sb.tile([128, HW], bf16, tag=f"xb{k}", name=f"xb{k}") for k in range(3)]
        vT_sb = [sb.tile([128, 1024], bf16, tag=f"vT{j}", name=f"vT{j}") for j in range(8)]
        vbT_sb = [sb.tile([128, 1024], bf16, tag=f"vbT{j}", name=f"vbT{j}") for j in range(8)]
        bias_sb = [sb.tile([1, 1024], bf16, tag=f"bias{i}", name=f"bias{i}") for i in range(6)]
        bo_sb = sb.tile([1, HIDDEN], bf16, tag="bo", name="bo")
        ones_sb = sb.tile([1, HW], bf16, tag="ones", name="ones")
        ident_sb = sb.tile([128, 128], bf16, tag="ident", name="ident")

        for k in range(3):
            nc.sync.dma_start(x_sb[k][:], x_d[128 * k : 128 * k + 128, :])
            nc.sync.dma_start(xb_sb[k][:], xb_d[128 * k : 128 * k + 128, :])
        for i in range(6):
            nc.sync.dma_start(bias_sb[i][:], bias_d[i : i + 1, :])
        nc.sync.dma_start(bo_sb[:], bo_d[:])
        nc.sync.dma_start(ident_sb[:], ident_d[:])
        nc.vector.memset(ones_sb[:], 1.0)

        # ---- P3..P5 outputs: num + den (~40KB/partition) ----
        post = ctx.enter_context(tc.tile_pool(name="post", bufs=1))
        num_sb = [post.tile([128, HW], bf16, tag=f"num{t}", name=f"num{t}") for t in range(16)]
        den_bf = [post.tile([32, HW], bf16, tag=f"denb{b}", name=f"denb{b}") for b in range(2)]
        den_f32 = [post.tile([32, HW], f32, tag=f"den{b}", name=f"den{b}") for b in range(2)]

        for _rep in range(reps):
            # ---- q/k conv outputs, live P1..P3 (~64KB/partition) ----
            with ExitStack() as qctx:
                qk = qctx.enter_context(tc.tile_pool(name="qk", bufs=1))
                q_sb = [qk.tile([128, HW], bf16, tag=f"q{g}", name=f"q{g}") for g in range(NGRP)]
                k_sb = [qk.tile([128, HW], bf16, tag=f"k{g}", name=f"k{g}") for g in range(NGRP)]
                qb_sb = [qk.tile([128, HW], bf16, tag=f"qb{g}", name=f"qb{g}") for g in range(NGRP)]
                kb_sb = [qk.tile([128, HW], bf16, tag=f"kb{g}", name=f"kb{g}") for g in range(NGRP)]

                def load_w(pool, wd, name):
                    tiles = [pool.tile([128, 1024], bf16, tag=f"{name}{k}", name=f"{name}{k}")
                             for k in range(3)]
                    for k in range(3):
                        nc.sync.dma_start(tiles[k][:], wd[128 * k : 128 * k + 128, :])
                    return tiles

                def conv_tile(wtiles, bias_row, src, g, mm):
                    for nhf in range(2):
                        sl = slice(512 * nhf, 512 * nhf + 512)
                        for k in range(3):
                            nc.tensor.matmul(
                                mm[:, sl], wtiles[k][:, 128 * g : 128 * g + 128],
                                src[k][:, sl], start=(k == 0), stop=False)
                        nc.tensor.matmul(
                            mm[:, sl], bias_sb[bias_row][0:1, 128 * g : 128 * g + 128],
                            ones_sb[0:1, sl], start=False, stop=True)

                # P1 wave 1: q, k from x
                with ExitStack() as pctx:
                    wsb = pctx.enter_context(tc.tile_pool(name="wsb1", bufs=1))
                    ps_conv = pctx.enter_context(
                        tc.tile_pool(name="ps_conv1", bufs=2, space="PSUM"))
                    wq_t = load_w(wsb, wq_d, "wq")
                    wk_t = load_w(wsb, wk_d, "wk")
                    for g in range(8):
                        mm = ps_conv.tile([128, HW], f32, tag="convmm", name="convmm")
                        conv_tile(wq_t, 0, x_sb, g, mm)
                        nc.vector.tensor_copy(q_sb[g][:], mm[:])
                    for g in range(8):
                        mm = ps_conv.tile([128, HW], f32, tag="convmm", name="convmm")
                        conv_tile(wk_t, 1, x_sb, g, mm)
                        nc.vector.tensor_copy(k_sb[g][:], mm[:])

                # P1 wave 2: q_bpf, k_bpf from x_bpf
                with ExitStack() as pctx:
                    wsb = pctx.enter_context(tc.tile_pool(name="wsb2", bufs=1))
                    ps_conv = pctx.enter_context(
                        tc.tile_pool(name="ps_conv2", bufs=2, space="PSUM"))
                    wqb_t = load_w(wsb, wqb_d, "wqb")
                    wkb_t = load_w(wsb, wkb_d, "wkb")
                    for g in range(8):
                        mm = ps_conv.tile([128, HW], f32, tag="convmm", name="convmm")
                        conv_tile(wqb_t, 2, xb_sb, g, mm)
                        nc.vector.tensor_copy(qb_sb[g][:], mm[:])
                    for g in range(8):
                        mm = ps_conv.tile([128, HW], f32, tag="convmm", name="convmm")
                        conv_tile(wkb_t, 3, xb_sb, g, mm)
                        nc.vector.tensor_copy(kb_sb[g][:], mm[:])

                # P2: v convs + transposes
                with ExitStack() as pctx:
                    wsb = pctx.enter_context(tc.tile_pool(name="wsb3", bufs=1))
                    ps_conv = pctx.enter_context(
                        tc.tile_pool(name="ps_conv3", bufs=2, space="PSUM"))
                    ps_tr = pctx.enter_context(
                        tc.tile_pool(name="ps_tr", bufs=2, space="PSUM"))
                    vstage = pctx.enter_context(tc.tile_pool(name="vstage", bufs=1))

                    def vconv_transpose(wtiles, bias_row, src, vT_tiles):
                        # wave w: conv out tiles tau in [4w,4w+4) -> vT cols 512w..+512
                        for w in range(2):
                            vtmp = [vstage.tile([128, 1024], bf16, tag=f"vs{t}", name=f"vs{t}")
                                    for t in range(4)]
                            for t in range(4):
                                g = 4 * w + t
                                mm = ps_conv.tile([128, HW], f32, tag="convmm", name="convmm")
                                conv_tile(wtiles, bias_row, src, g, mm)
                                nc.vector.tensor_copy(vtmp[t][:], mm[:])
                            for j in range(8):
                                tp = ps_tr.tile([128, 512], bf16, tag="tp", name="tp")
                                for t in range(4):
                                    nc.tensor.transpose(
                                        tp[:, 128 * t : 128 * t + 128],
                                        vtmp[t][:, 128 * j : 128 * j + 128], ident_sb[:])
                                nc.vector.tensor_copy(
                                    vT_tiles[j][:, 512 * w : 512 * w + 512], tp[:])
                        for j in range(8):
                            # ones column at dp=12 of every head -> denominator channel
                            nc.vector.memset(vT_tiles[j][:, 12:1024:32], 1.0)

                    wv_t = load_w(wsb, wv_d, "wv")
                    wvb_t = load_w(wsb, wvb_d, "wvb")
                    vconv_transpose(wv_t, 4, x_sb, vT_sb)
                    vconv_transpose(wvb_t, 5, xb_sb, vbT_sb)

                # ---- P3: attention ----
                with ExitStack() as pctx:
                    ps_sc = pctx.enter_context(tc.tile_pool(name="ps_sc", bufs=2, space="PSUM"))
                    ps_av = pctx.enter_context(tc.tile_pool(name="ps_av", bufs=2, space="PSUM"))
                    peT = pctx.enter_context(tc.tile_pool(name="peT", bufs=3))

                    for br in range(2):
                        # br0: f = mha(q_bpf, k, v);  br1: f_bpf = mha(q, k_bpf, v_bpf)
                        qs = qb_sb if br == 0 else q_sb
                        ks = k_sb if br == 0 else kb_sb
                        vTs = vT_sb if br == 0 else vbT_sb
                        for grp in range(NGRP):
                            t16 = 8 * br + grp
                            for nh in range(2):
                                av = ps_av.tile([128, 512], f32, tag="av", name="av")
                                for m in range(8):
                                    for p in range(2):
                                        s2 = ps_sc.tile([128, 1024], f32, tag="s2", name="s2")
                                        for cl in range(2):
                                            c = 2 * p + cl
                                            nc.tensor.matmul(
                                                s2[:, 512 * cl : 512 * cl + 512],
                                                ks[grp][32 * c : 32 * c + 16, 128 * m : 128 * m + 128],
                                                qs[grp][32 * c : 32 * c + 16, 512 * nh : 512 * nh + 512],
                                                start=True, stop=True, tile_position=(32 * c, 0))
                                        eT = peT.tile([128, 1024], bf16, tag="eT", name="eT")
                                        nc.scalar.activation(
                                            eT[:], s2[:], mybir.ActivationFunctionType.Exp)
                                        for cl in range(2):
                                            c = 2 * p + cl
                                            h = 4 * grp + c
                                            nc.tensor.matmul(
                                                av[32 * c : 32 * c + 32, :],
                                                vTs[m][:, 32 * h : 32 * h + 32],
                                                eT[:, 512 * cl : 512 * cl + 512],
                                                start=(m == 0), stop=(m == 7),
                                                tile_position=(0, 32 * c),
                                                skip_group_check=True)
                                # numerators (bf16); denominator rows gathered by DMA
                                # (compute engines cannot read strided partitions)
                                nc.vector.tensor_copy(
                                    num_sb[t16][:, 512 * nh : 512 * nh + 512], av[:])
                                for c in range(4):
                                    nc.sync.dma_start(
                                        den_bf[br][4 * grp + c : 4 * grp + c + 1,
                                                   512 * nh : 512 * nh + 512],
                                        num_sb[t16][32 * c + 12 : 32 * c + 13,
                                                    512 * nh : 512 * nh + 512])

            # ---- P4: normalize (q/k pool closed above frees SBUF) ----
            with ExitStack() as pctx:
                p4 = pctx.enter_context(tc.tile_pool(name="p4", bufs=1))
                ps_rdb = pctx.enter_context(tc.tile_pool(name="ps_rdb", bufs=2, space="PSUM"))
                rd_f32 = [p4.tile([32, HW], f32, tag=f"rd{b}", name=f"rd{b}") for b in range(2)]
                sel_sb = [p4.tile([32, 128], f32, tag=f"sel{g}", name=f"sel{g}") for g in range(NGRP)]
                for g in range(NGRP):
                    nc.sync.dma_start(sel_sb[g][:], sel_d[g])
                for br in range(2):
                    nc.vector.tensor_copy(den_f32[br][:], den_bf[br][:])
                    nc.vector.reciprocal(rd_f32[br][:], den_f32[br][:])
                for br in range(2):
                    for grp in range(NGRP):
                        t16 = 8 * br + grp
                        rdb = ps_rdb.tile([128, HW], f32, tag="rdb", name="rdb")
                        for nh in range(2):
                            nc.tensor.matmul(
                                rdb[:, 512 * nh : 512 * nh + 512], sel_sb[grp][:],
                                rd_f32[br][:, 512 * nh : 512 * nh + 512],
                                start=True, stop=True)
                        nc.vector.tensor_mul(num_sb[t16][:], num_sb[t16][:], rdb[:])

            # ---- P5: output conv (+ residual terms over x/x_bpf) ----
            with ExitStack() as pctx:
                p5 = pctx.enter_context(tc.tile_pool(name="p5", bufs=1))
                ps_out = pctx.enter_context(tc.tile_pool(name="ps_out", bufs=2, space="PSUM"))
                po = pctx.enter_context(tc.tile_pool(name="po", bufs=2))
                wo_sb = [p5.tile([128, HIDDEN], bf16, tag=f"wo{t}", name=f"wo{t}") for t in range(16)]
                wores_sb = [p5.tile([128, HIDDEN], bf16, tag=f"wor{k}", name=f"wor{k}") for k in range(6)]
                for t in range(16):
                    nc.sync.dma_start(wo_sb[t][:], wo_d[128 * t : 128 * t + 128, :])
                for k in range(6):
                    nc.sync.dma_start(wores_sb[k][:], wores_d[128 * k : 128 * k + 128, :])
                for tau in range(4):
                    mm = ps_out.tile([128, HW], f32, tag="omm", name="omm")
                    for nhf in range(2):
                        sl = slice(512 * nhf, 512 * nhf + 512)
                        for t16 in range(16):
                            nc.tensor.matmul(
                                mm[:, sl], wo_sb[t16][:, 128 * tau : 128 * tau + 128],
                                num_sb[t16][:, sl], start=(t16 == 0), stop=False)
                        for k in range(6):
                            rhs = x_sb[k] if k < 3 else xb_sb[k - 3]
                            nc.tensor.matmul(
                                mm[:, sl], wores_sb[k][:, 128 * tau : 128 * tau + 128],
                                rhs[:, sl], start=False, stop=False)
                        nc.tensor.matmul(
                            mm[:, sl], bo_sb[0:1, 128 * tau : 128 * tau + 128],
                            ones_sb[0:1, sl], start=False, stop=True)
                    o_sb = po.tile([128, HW], f32, tag="osb", name="osb")
                    nc.vector.tensor_copy(o_sb[:], mm[:])
                    nc.sync.dma_start(out_d[128 * tau : 128 * tau + 128, :], o_sb[:])

    nc.compile()
    return nc


def prep_weights(Wq, bq, Wk, bk, Wv, bv, Wq_bpf, bq_bpf, Wk_bpf, bk_bpf,
                 Wv_bpf, bv_bpf, Wo, bo):
    """Host-side channel shuffles + scale folding. Returns dict of arrays."""
    def qk_pad(Wm, bias, scale):
        wt = np.zeros((EMBED, 1024), np.float32)
        bp = np.zeros((1024,), np.float32)
        for h in range(NH):
            r0 = _qk_row(h, 0)
            wt[:, r0 : r0 + D] = (Wm[12 * h : 12 * h + 12, :] * scale).T
            bp[r0 : r0 + D] = bias[12 * h : 12 * h + 12] * scale
        return wt, bp

    def v_pad(Wm, bias):
        wt = np.zeros((EMBED, 1024), np.float32)
        bp = np.zeros((1024,), np.float32)
        for h in range(NH):
            r0 = _v_col(h, 0)
            wt[:, r0 : r0 + D] = Wm[12 * h : 12 * h + 12, :].T
            bp[r0 : r0 + D] = bias[12 * h : 12 * h + 12]
        return wt, bp

    wq_t, bq_p = qk_pad(Wq, bq, SCALE)        # branch-2 query (scaled)
    wk_t, bk_p = qk_pad(Wk, bk, 1.0)
    wqb_t, bqb_p = qk_pad(Wq_bpf, bq_bpf, SCALE)  # branch-1 query (scaled)
    wkb_t, bkb_p = qk_pad(Wk_bpf, bk_bpf, 1.0)
    wv_t, bv_p = v_pad(Wv, bv)
    wvb_t, bvb_p = v_pad(Wv_bpf, bv_bpf)

    biases = np.stack([bq_p, bk_p, bqb_p, bkb_p, bv_p, bvb_p])  # [6, 1024]

    # output conv over padded fused channels
    wo_big = np.zeros((2048, HIDDEN), np.float32)
    for br in range(2):
        for h in range(NH):
            grp, c = h // 4, h % 4
            row0 = 128 * (8 * br + grp) + 32 * c
            col0 = EMBED * br + 12 * h
            wo_big[row0 : row0 + D, :] = Wo[:, col0 : col0 + D].T
    wo_res = Wo.T.copy()  # [768, 512]; rows 0-383 pair with x, 384-767 with x_bpf

    sel = np.zeros((NGRP, 32, 128), np.float32)
    for grp in range(NGRP):
        for c in range(4):
            h = 4 * grp + c
            for dp in range(D):
                sel[grp, h, 32 * c + dp] = 1.0

    bf = ml_dtypes.bfloat16
    return {
        "wq": wq_t.astype(bf), "wk": wk_t.astype(bf),
        "wqb": wqb_t.astype(bf), "wkb": wkb_t.astype(bf),
        "wv": wv_t.astype(bf), "wvb": wvb_t.astype(bf),
        "biases": biases.astype(bf),
        "wo": wo_big.astype(bf), "wores": wo_res.astype(bf),
        "bo": bo.reshape(1, HIDDEN).astype(bf),
        "sel": sel,
        "ident": np.eye(128).astype(bf),
    }


_NC = None


def kernel(x, x_bpf, Wq, bq, Wk, bk, Wv, bv, Wq_bpf, bq_bpf, Wk_bpf, bk_bpf,
           Wv_bpf, bv_bpf, Wo, bo, _trace=False):
    global _NC
    if _NC is None:
        _NC = build_program()
    nc = _NC

    shared = prep_weights(np.asarray(Wq, np.float32), np.asarray(bq, np.float32),
                          np.asarray(Wk, np.float32), np.asarray(bk, np.float32),
                          np.asarray(Wv, np.float32), np.asarray(bv, np.float32),
                          np.asarray(Wq_bpf, np.float32), np.asarray(bq_bpf, np.float32),
                          np.asarray(Wk_bpf, np.float32), np.asarray(bk_bpf, np.float32),
                          np.asarray(Wv_bpf, np.float32), np.asarray(bv_bpf, np.float32),
                          np.asarray(Wo, np.float32), np.asarray(bo, np.float32))

    bf = ml_dtypes.bfloat16
    x = np.asarray(x, np.float32).reshape(B, EMBED, HW)
    x_bpf = np.asarray(x_bpf, np.float32).reshape(B, EMBED, HW)
    in_maps = []
    for b in range(B):
        m = dict(shared)
        m["x"] = x[b].astype(bf)
        m["xb"] = x_bpf[b].astype(bf)
        in_maps.append(m)

    res = run_bass_kernel_spmd(nc, in_maps, list(range(NCORES)), trace=_trace)
    out = np.stack([res.results[b]["out"] for b in range(B)])  # [8, 512, 1024]
    out = out.reshape(B, HIDDEN, H, W).astype(np.float32)
    if _trace:
        return out, res
    return out

